# revision 1
# baseline (speedup 1.0000x reference)
"""Dense-CRF mean-field inference on 8 Trainium2 NeuronCores.

Math restructuring (validated numerically against the jax reference):
  - Kb + Kg share weight 1.0 -> single kernel matrix K = exp(-.5 d2_b) + exp(-.5 d2_g).
  - The Potts 3x3 conv update is  upd[c] = boxsum3(S) - boxsum3(comb[c]) with
    S = sum_c comb[c]; the S part is class-independent so softmax drops it:
        out = softmax(input + UPDATE_FACTOR * boxsum3(comb[c])).
    The UPDATE_FACTOR (3.0) is folded into K via exp(x + ln 3).
  - Spatial sigma 5 -> K decays fast with |dy|; rows further than ~20 image rows
    from the output pixel contribute < 1e-5 relative.  Each core keeps a
    41-block (5248 px) band of K rows resident in SBUF: blocks within +-6 rows
    in fp32, the rest fp16 (validated: l2 rel err 2.9e-5 vs fp32-exact 2.2e-5).
  - -0.5*||fi-fj||^2 is computed by ONE matmul per kernel via augmented
    features: G=[y,x,-.5|s|^2,1,r,g,b,-.5|c|^2,1], H=[y,x,1,-.5|s|^2,r,g,b,1,-.5|c|^2];
    gaussian = rows 0:4, bilateral = rows 0:9.
  - Each core computes comb for 14 image rows (its 12 + 1 halo row each side,
    edge rows duplicated via clamped features) so the 3x3 conv is local.
    One AllGather of the new per-core probabilities per iteration.

Sharding: core r owns output image rows [12r, 12r+12); K band = global
128-px blocks [9r-16, 9r+25) (zero-K padding outside the image).
"""

import os
import sys

import numpy as np

for _p in ("/opt/trn_rl_repo",):
    if _p not in sys.path and os.path.isdir(_p):
        sys.path.insert(0, _p)

H = 96
W = 96
C = 5
N = H * W                      # 9216
NCORES = 8
RPC = H // NCORES              # 12 image rows per core
NLOC = (RPC + 2) * W           # 1344 extended-output pixels (14 rows)
NMID = RPC * W                 # 1152 owned pixels
BLK = 128
NBLK = 41                      # K band m-blocks per core
BAND_LO = -16                  # band start, in global blocks, relative to 9r
F32_LO, F32_HI = 12, 29        # band-local block range kept in fp32 (+-4 rows)
N32 = F32_HI - F32_LO          # 21 fp32 blocks
N16 = NBLK - N32               # 20 fp16 blocks
GBLK = N // BLK                # 72 global blocks
PADBLK = 16                    # padding blocks each side of flat_padded
FPW = (GBLK + 2 * PADBLK) * C  # flat_padded free width = 520
CH = 448                       # matvec/exp n-chunk (fits one PSUM bank)
NCH = 3
ITERS = 5
LN3 = float(np.log(3.0))
NEG = -1.0e30                  # kills exp() for out-of-image padding blocks

_CACHED_NC = None


def _near(i):
    return F32_LO <= i < F32_HI


def _k16_idx(i):
    return i if i < F32_LO else i - N32


def _build_module():
    import concourse.bass as bass
    import concourse.bacc as bacc
    import concourse.tile as tile
    from concourse import mybir
    from concourse.masks import make_identity

    f32 = mybir.dt.float32
    f16 = mybir.dt.float16
    u32 = mybir.dt.uint32
    EXP = mybir.ActivationFunctionType.Exp
    COPY = mybir.ActivationFunctionType.Copy

    nc = bacc.Bacc("TRN2", target_bir_lowering=False, debug=False,
                   num_devices=NCORES)

    g_dram = nc.dram_tensor("g_feats", [9, NBLK * BLK], f32, kind="ExternalInput")
    h_dram = nc.dram_tensor("h_feats", [9, NLOC], f32, kind="ExternalInput")
    ipp_dram = nc.dram_tensor("inp_pp", [BLK, GBLK * C], f32, kind="ExternalInput")
    icn_dram = nc.dram_tensor("inp_cn", [C, NMID], f32, kind="ExternalInput")
    boff_dram = nc.dram_tensor("band_off", [1, 1], u32, kind="ExternalInput")
    kg32_dram = nc.dram_tensor("kg32", [BLK, N32 * NCH * CH], f32,
                               kind="ExternalInput")
    kg16_dram = nc.dram_tensor("kg16", [BLK, N16 * NCH * CH], f16,
                               kind="ExternalInput")
    out_dram = nc.dram_tensor("out_loc", [BLK, (NMID // BLK) * C], f32,
                              kind="ExternalOutput")

    def bcast_inner(ap, n):
        return bass.AP(tensor=ap.tensor, offset=ap.offset, ap=[*ap.ap, [0, n]])

    with tile.TileContext(nc) as tc:
        with tc.tile_pool(name="singles", bufs=1) as singles, \
             tc.tile_pool(name="warmps", bufs=1, space="PSUM") as warmpool, \
             tc.tile_pool(name="dram", bufs=1, space="DRAM") as dram:

            # ---- long-lived SBUF state ----
            k32 = singles.tile([BLK, N32, NCH * CH], f32, name="k32")
            k16 = singles.tile([BLK, N16, NCH * CH], f16, name="k16")
            flat_pad = singles.tile([BLK, FPW], f32, name="flat_pad")
            h_sb = singles.tile([9, NLOC], f32, name="h_sb")
            ipp_sb = singles.tile([BLK, GBLK * C], f32, name="ipp_sb")
            icn_sb = singles.tile([C, NMID], f32, name="icn_sb")
            ident = singles.tile([BLK, BLK], f32, name="ident")
            boff_sb = singles.tile([1, 1], u32, name="boff_sb")
            ln3_sb = singles.tile([BLK, 1], f32, name="ln3_sb")
            nc.vector.memset(ln3_sb, LN3)
            # HAM warm-keeper: dummy matmuls that fill PE-idle windows so the
            # activity monitor keeps the PE clock at 2.4 GHz (it halves the
            # clock after ~3.4us of idle).  ~426 ns each (fp32 512-col).
            warm_ps = warmpool.tile([1, 512], f32, name="warm_ps")

            def warm(n):
                for _ in range(n):
                    nc.tensor.matmul(warm_ps, ident[:, 0:1], k32[:, 0, 0:512],
                                     start=True, stop=True)

            ag_in = dram.tile([BLK, (NMID // BLK) * C], f32, name="ag_in")
            ag_out = dram.tile([BLK * NCORES, (NMID // BLK) * C], f32, name="ag_out")

            nc.sync.dma_start(out=h_sb, in_=h_dram[:, :])
            nc.sync.dma_start(out=ipp_sb, in_=ipp_dram[:, :])
            nc.sync.dma_start(out=icn_sb, in_=icn_dram[:, :])
            nc.sync.dma_start(out=boff_sb, in_=boff_dram[:, :])
            make_identity(nc, ident)
            nc.vector.memset(flat_pad, 0.0)

            # band offset register (elements into flat_pad) = 45 * core_id
            boff_regs = nc.alloc_registers("boff_regs",
                                           engines=(mybir.EngineType.DVE,))
            nc.regs_load(boff_regs, boff_sb[0:1, 0:1])
            off_sv = nc.snap(boff_regs, donate=True, min_val=0,
                             max_val=(NCORES - 1) * 9 * C)

            # ---- phase 1: build K band ----
            # Bilateral part on device (input-dependent); the gaussian part is
            # input-independent so the host ships it precomputed (kg32/kg16)
            # and we just add it.
            with tc.tile_pool(name="gstage", bufs=3) as gpool, \
                 tc.tile_pool(name="kgstage", bufs=3) as kgpool, \
                 tc.tile_pool(name="bpsum", bufs=2, space="PSUM") as bppool:
                for i in range(NBLK):
                    gt = gpool.tile([9, BLK], f32, tag="gt")
                    nc.sync.dma_start(out=gt, in_=g_dram[:, i * BLK:(i + 1) * BLK])
                    if _near(i):
                        kdst = k32[:, i - F32_LO, :]
                        kdt = f32
                        j = i - F32_LO
                        kg_src = kg32_dram[:, j * NCH * CH:(j + 1) * NCH * CH]
                    else:
                        kdst = k16[:, _k16_idx(i), :]
                        kdt = f16
                        j = _k16_idx(i)
                        kg_src = kg16_dram[:, j * NCH * CH:(j + 1) * NCH * CH]
                    kg = kgpool.tile([BLK, NCH * CH], kdt, tag="kg")
                    nc.sync.dma_start(out=kg, in_=kg_src)
                    pb = bppool.tile([BLK, NCH, 512], f32, tag="pb")
                    for nb in range(NCH):
                        hs = h_sb[:, nb * CH:(nb + 1) * CH]
                        nc.tensor.matmul(pb[:, nb, 0:CH], gt[0:9, :], hs[0:9, :],
                                         start=True, stop=True)
                    kv = kdst.rearrange("p (a c) -> p a c", c=CH)
                    nc.scalar.activation(out=kv, in_=pb[:, :, 0:CH], func=EXP,
                                         bias=ln3_sb)
                    nc.vector.tensor_add(kdst, kdst, kg)
                warm(12)

            # ---- helpers ----
            def softmax_pp(pool, u_pp, mb, tag):
                """u_pp: [128, mb*C] logits, pixel-partition layout -> probs."""
                v = u_pp.rearrange("p (a c) -> p a c", c=C)
                mx = pool.tile([BLK, mb], f32, tag=f"{tag}_mx")
                nc.vector.tensor_reduce(out=mx, in_=v,
                                        axis=mybir.AxisListType.X,
                                        op=mybir.AluOpType.max)
                e = pool.tile([BLK, mb * C], f32, tag=f"{tag}_e")
                ev = e.rearrange("p (a c) -> p a c", c=C)
                nc.vector.tensor_sub(ev, v, bcast_inner(mx, C))
                nc.scalar.activation(out=e, in_=e, func=EXP)
                s = pool.tile([BLK, mb], f32, tag=f"{tag}_s")
                nc.vector.tensor_reduce(out=s, in_=ev,
                                        axis=mybir.AxisListType.X,
                                        op=mybir.AluOpType.add)
                nc.vector.reciprocal(out=s, in_=s)
                fl = pool.tile([BLK, mb * C], f32, tag=f"{tag}_fl")
                nc.vector.tensor_mul(fl.rearrange("p (a c) -> p a c", c=C), ev,
                                     bcast_inner(s, C))
                return fl

            # ---- phase 2: initial flat = softmax(input) ----
            with tc.tile_pool(name="init", bufs=1) as ipool:
                fl0 = softmax_pp(ipool, ipp_sb, GBLK, "sm0")
                nc.vector.tensor_copy(
                    out=flat_pad[:, PADBLK * C:(PADBLK + GBLK) * C], in_=fl0)

            # ---- phase 3: iterations ----
            with tc.tile_pool(name="iter", bufs=1) as wpool, \
                 tc.tile_pool(name="band", bufs=2) as bpool, \
                 tc.tile_pool(name="smx", bufs=2) as spool, \
                 tc.tile_pool(name="ipsum", bufs=2, space="PSUM") as ippool:
                for it in range(ITERS):
                    band32 = bpool.tile([BLK, NBLK * C], f32, tag="band32")
                    nc.vector.tensor_copy(
                        out=band32, in_=flat_pad[:, bass.ds(off_sv, NBLK * C)])
                    band16 = bpool.tile([BLK, NBLK * C], f16, tag="band16")
                    nc.vector.tensor_copy(out=band16, in_=band32)

                    # matvec: comb[c, n] = sum_m K[m, n] * flat[c, m]
                    pv = ippool.tile([C, NCH, 512], f32, tag="pv", bufs=1)
                    for nb in range(NCH):
                        for i in range(NBLK):
                            if _near(i):
                                lhs = band32[:, i * C:(i + 1) * C]
                                kt = k32[:, i - F32_LO, nb * CH:(nb + 1) * CH]
                            else:
                                lhs = band16[:, i * C:(i + 1) * C]
                                kt = k16[:, _k16_idx(i), nb * CH:(nb + 1) * CH]
                            nc.tensor.matmul(pv[:, nb, 0:CH], lhs, kt,
                                             start=(i == 0), stop=(i == NBLK - 1))
                    warm(20)
                    comb = wpool.tile([C, NLOC], f32, tag="comb")
                    nc.scalar.activation(
                        out=comb.rearrange("p (a c) -> p a c", c=CH),
                        in_=pv[:, :, 0:CH], func=COPY)

                    # 3x3 box sum: x-pass into t1 (all 14 rows), edge-replicated
                    t1 = wpool.tile([C, NLOC], f32, tag="t1")
                    nc.vector.tensor_add(t1[:, 1:NLOC - 1], comb[:, 0:NLOC - 2],
                                         comb[:, 2:NLOC])
                    nc.vector.tensor_add(t1[:, 1:NLOC - 1], t1[:, 1:NLOC - 1],
                                         comb[:, 1:NLOC - 1])
                    t1r = t1.rearrange("p (row x) -> p row x", x=W)
                    cbr = comb.rearrange("p (row x) -> p row x", x=W)
                    # x = 0 column: 2*c[0] + c[1]
                    nc.vector.tensor_add(t1r[:, :, 0:1], cbr[:, :, 0:1],
                                         cbr[:, :, 1:2])
                    nc.vector.tensor_add(t1r[:, :, 0:1], t1r[:, :, 0:1],
                                         cbr[:, :, 0:1])
                    # x = W-1 column: c[W-2] + 2*c[W-1]
                    nc.vector.tensor_add(t1r[:, :, W - 1:W], cbr[:, :, W - 2:W - 1],
                                         cbr[:, :, W - 1:W])
                    nc.vector.tensor_add(t1r[:, :, W - 1:W], t1r[:, :, W - 1:W],
                                         cbr[:, :, W - 1:W])
                    # y-pass (middle 12 rows) + input logits
                    u = wpool.tile([C, NMID], f32, tag="u")
                    nc.vector.tensor_add(u, t1[:, 0:NMID], t1[:, 2 * W:NLOC])
                    nc.vector.tensor_add(u, u, t1[:, W:NMID + W])
                    nc.vector.tensor_add(u, u, icn_sb)

                    # transpose U [5, 1152] -> pixel-partition [128, 9*5]
                    u_pp = spool.tile([BLK, (NMID // BLK) * C], f32, tag="u_pp")
                    for kb in range(NMID // BLK):
                        pt = ippool.tile([BLK, C], f32, tag="pt")
                        nc.tensor.transpose(pt, u[:, kb * BLK:(kb + 1) * BLK],
                                            ident[0:C, 0:C])
                        nc.vector.tensor_copy(out=u_pp[:, kb * C:(kb + 1) * C],
                                              in_=pt)

                    flat_l = softmax_pp(spool, u_pp, NMID // BLK, "smx")
                    if it < ITERS - 1:
                        warm(42)

                    if it < ITERS - 1:
                        nc.sync.dma_start(out=ag_in, in_=flat_l)
                        nc.gpsimd.collective_compute(
                            "AllGather",
                            mybir.AluOpType.bypass,
                            replica_groups=[list(range(NCORES))],
                            ins=[ag_in.opt()],
                            outs=[ag_out.opt()],
                        )
                        nc.sync.dma_start(
                            out=flat_pad[:, PADBLK * C:(PADBLK + GBLK) * C]
                            .rearrange("p (r j) -> p r j", r=NCORES),
                            in_=ag_out.rearrange("(r p) j -> p r j", p=BLK))
                    else:
                        nc.sync.dma_start(out=out_dram[:, :], in_=flat_l)

    nc.compile()
    return nc


def _host_inputs(input_tensor, reference_tensor):
    logits = np.ascontiguousarray(
        np.asarray(input_tensor, dtype=np.float32)[0].reshape(C, N))
    ref = np.asarray(reference_tensor, dtype=np.float32)[0]  # [3, 96, 96]

    yy, xx = np.meshgrid(np.arange(H, dtype=np.float32),
                         np.arange(W, dtype=np.float32), indexing="ij")
    Y = (yy / 5.0).reshape(N)
    X = (xx / 5.0).reshape(N)
    RGB = (ref / 0.5).reshape(3, N)
    s2 = -0.5 * (Y * Y + X * X)
    c2 = -0.5 * (RGB * RGB).sum(axis=0)
    ones = np.ones(N, np.float32)

    # G (band / m side) and H (output / n side) augmented features
    G_all = np.stack([Y, X, s2, ones, RGB[0], RGB[1], RGB[2], c2, ones])
    H_all = np.stack([Y, X, ones, s2, RGB[0], RGB[1], RGB[2], ones, c2])

    # input in pixel-partition layout [128, 72*5]
    ipp = np.ascontiguousarray(
        logits.reshape(C, GBLK, BLK).transpose(2, 1, 0).reshape(BLK, GBLK * C))

    # gaussian kernel tables: 3*exp(-(dy^2+dx^2)/50), folded update factor 3
    dtab = np.exp(-(np.arange(-(H - 1), H) ** 2) / 50.0).astype(np.float64)
    gx3 = (3.0 * dtab).astype(np.float32)
    gy1 = dtab.astype(np.float32)
    yy_all = (np.arange(N) // W).astype(np.int64)
    xx_all = (np.arange(N) % W).astype(np.int64)

    def kg_for_core(r, yn, xn):
        """[NBLK, 128, 1344] gaussian kernel values for core r's band."""
        kg = np.zeros((NBLK, BLK, NLOC), np.float32)
        for i in range(NBLK):
            gb = 9 * r + BAND_LO + i
            if 0 <= gb < GBLK:
                pm = np.arange(gb * BLK, (gb + 1) * BLK)
                A = gy1[yy_all[pm][:, None] - yn[None, :] + H - 1]
                B = gx3[xx_all[pm][:, None] - xn[None, :] + H - 1]
                kg[i] = A * B
        return kg

    in_maps = []
    kg_interior = None
    for r in range(NCORES):
        g = np.zeros((9, NBLK * BLK), np.float32)
        g[2, :] = NEG
        for i in range(NBLK):
            gb = 9 * r + BAND_LO + i
            if 0 <= gb < GBLK:
                g[:, i * BLK:(i + 1) * BLK] = G_all[:, gb * BLK:(gb + 1) * BLK]
        yext = np.clip(np.arange(RPC * r - 1, RPC * (r + 1) + 1), 0, H - 1)
        hpix = (yext[:, None] * W + np.arange(W)[None, :]).reshape(-1)
        h = np.ascontiguousarray(H_all[:, hpix])
        icn = np.ascontiguousarray(
            logits.reshape(C, H, W)[:, RPC * r:RPC * (r + 1), :].reshape(C, NMID))
        # gaussian part of K (interior cores share one array)
        if 2 <= r <= 5:
            if kg_interior is None:
                kg_interior = kg_for_core(r, yy_all[hpix], xx_all[hpix])
            kg = kg_interior
        else:
            kg = kg_for_core(r, yy_all[hpix], xx_all[hpix])
        near_idx = list(range(F32_LO, F32_HI))
        far_idx = [i for i in range(NBLK) if not _near(i)]
        far_idx = sorted(far_idx, key=_k16_idx)
        kg32 = np.ascontiguousarray(
            kg[near_idx].transpose(1, 0, 2).reshape(BLK, N32 * NLOC))
        kg16 = np.ascontiguousarray(
            kg[far_idx].transpose(1, 0, 2).reshape(BLK, N16 * NLOC)
        ).astype(np.float16)
        in_maps.append({
            "g_feats": g,
            "h_feats": h,
            "inp_pp": ipp,
            "inp_cn": icn,
            "band_off": np.array([[9 * C * r]], np.uint32),
            "kg32": kg32,
            "kg16": kg16,
        })
    return in_maps


def _assemble(results):
    out = np.empty((C, N), np.float32)
    for r in range(NCORES):
        blk = results[r]["out_loc"].reshape(BLK, NMID // BLK, C)
        out[:, NMID * r:NMID * (r + 1)] = (
            blk.transpose(2, 1, 0).reshape(C, NMID))
    return out.reshape(1, C, H, W)


def _get_nc():
    global _CACHED_NC
    if _CACHED_NC is None:
        _CACHED_NC = _build_module()
    return _CACHED_NC


def run(input_tensor, reference_tensor, trace=False):
    from concourse.bass_utils import run_bass_kernel_spmd
    nc = _get_nc()
    in_maps = _host_inputs(input_tensor, reference_tensor)
    res = run_bass_kernel_spmd(nc, in_maps, core_ids=list(range(NCORES)),
                               trace=trace)
    return _assemble(res.results), res


def kernel(input_tensor, reference_tensor):
    out, _ = run(input_tensor, reference_tensor, trace=False)
    return out



# revision 3
# speedup vs baseline: 1.4682x; 1.4682x over previous
"""Dense-CRF mean-field inference on 8 Trainium2 NeuronCores.

Math restructuring (validated numerically against the jax reference):
  - Kb + Kg share weight 1.0 -> single kernel matrix K = exp(-.5 d2_b) + exp(-.5 d2_g).
  - The Potts 3x3 conv update is  upd[c] = boxsum3(S) - boxsum3(comb[c]) with
    S = sum_c comb[c]; the S part is class-independent so softmax drops it:
        out = softmax(input + UPDATE_FACTOR * boxsum3(comb[c])).
    The UPDATE_FACTOR (3.0) is folded into K via exp(x + ln 3).
  - Spatial sigma 5 -> K decays fast with |dy|; each core keeps a 41-block
    (5248 px) band of K rows resident in SBUF, all fp16 (validated err 1.1e-4
    on the real inputs; fp32 matmuls run dual-pass LOW_HIGH at 2.3x the cost,
    so fp16 everywhere doubles TensorE throughput).
  - -0.5*||fi-fj||^2 is computed by ONE matmul per kernel via augmented
    features: G=[y,x,-.5|s|^2,1,r,g,b,-.5|c|^2,1], H=[y,x,1,-.5|s|^2,r,g,b,1,-.5|c|^2];
    the gaussian part is input-independent and host-shipped in fp16.
  - Each core computes comb for 14 image rows (its 12 + 1 halo row each side,
    edge rows duplicated via clamped features) so the 3x3 conv is local.
    One AllGather of the new per-core probabilities per iteration; the next
    iteration's matvec runs its 9 central (own-pixel) K blocks first straight
    off the local softmax output, hiding most of the AllGather latency.

Sharding: core r owns output image rows [12r, 12r+12); K band = global
128-px blocks [9r-16, 9r+25) (zero-K padding outside the image).
"""

import os
import sys

import numpy as np

for _p in ("/opt/trn_rl_repo",):
    if _p not in sys.path and os.path.isdir(_p):
        sys.path.insert(0, _p)

H = 96
W = 96
C = 5
N = H * W                      # 9216
NCORES = 8
RPC = H // NCORES              # 12 image rows per core
NLOC = (RPC + 2) * W           # 1344 extended-output pixels (14 rows)
NMID = RPC * W                 # 1152 owned pixels
BLK = 128
NBLK = 41                      # K band m-blocks per core
BAND_LO = -16                  # band start, in global blocks, relative to 9r
CEN_LO = -BAND_LO              # band-local index of first central (own) block
GBLK = N // BLK                # 72 global blocks
PADBLK = -BAND_LO              # padding blocks each side of flat_padded
FPW = (GBLK + 2 * PADBLK) * C  # flat_padded free width
CH = 448                       # matvec/exp n-chunk (fits one PSUM bank)
NCH = 3
ITERS = 5
LN3 = float(np.log(3.0))
NEG = -1.0e30                  # kills exp() for out-of-image padding blocks

_CACHED_NC = None


def _build_module():
    import concourse.bass as bass
    import concourse.bacc as bacc
    import concourse.tile as tile
    from concourse import mybir
    from concourse.masks import make_identity

    f32 = mybir.dt.float32
    f16 = mybir.dt.float16
    u32 = mybir.dt.uint32
    EXP = mybir.ActivationFunctionType.Exp
    COPY = mybir.ActivationFunctionType.Copy

    nc = bacc.Bacc("TRN2", target_bir_lowering=False, debug=False,
                   num_devices=NCORES)

    g_dram = nc.dram_tensor("g_feats", [9, NBLK * BLK], f32, kind="ExternalInput")
    h_dram = nc.dram_tensor("h_feats", [9, NLOC], f32, kind="ExternalInput")
    ipp_dram = nc.dram_tensor("inp_pp", [BLK, GBLK * C], f32, kind="ExternalInput")
    icn_dram = nc.dram_tensor("icn_pp", [BLK, 9 * C], f32, kind="ExternalInput")
    boff_dram = nc.dram_tensor("band_off", [1, 2], u32, kind="ExternalInput")
    kg_dram = nc.dram_tensor("kg16", [BLK, NBLK * NLOC], f16, kind="ExternalInput")
    out_dram = nc.dram_tensor("out_loc", [BLK, 9 * C], f32, kind="ExternalOutput")

    def bcast_inner(ap, n):
        return bass.AP(tensor=ap.tensor, offset=ap.offset, ap=[*ap.ap, [0, n]])

    with tile.TileContext(nc) as tc:
        with tc.tile_pool(name="singles", bufs=1) as singles, \
             tc.tile_pool(name="warmps", bufs=1, space="PSUM") as warmpool, \
             tc.tile_pool(name="dram", bufs=1, space="DRAM") as dram:

            # ---- long-lived SBUF state ----
            k16 = singles.tile([BLK, NBLK, NCH * CH], f16, name="k16")
            flat_pad = singles.tile([BLK, FPW], f32, name="flat_pad")
            g_sb = singles.tile([9, NBLK * BLK], f32, name="g_sb")
            h_sb = singles.tile([9, NLOC], f32, name="h_sb")
            ipp_sb = singles.tile([BLK, GBLK * C], f32, name="ipp_sb")
            icn_sb = singles.tile([BLK, 9 * C], f32, name="icn_sb")
            ident = singles.tile([BLK, BLK], f32, name="ident")
            boff_sb = singles.tile([1, 2], u32, name="boff_sb")
            ln3_sb = singles.tile([BLK, 1], f32, name="ln3_sb")
            warm16 = singles.tile([BLK, 1], f16, name="warm16")
            nc.vector.memset(ln3_sb, LN3)
            nc.vector.memset(warm16, 0.0)
            # HAM warm-keeper: dummy fp16 matmuls bridge PE-idle windows so the
            # activity monitor keeps the PE clock at 2.4 GHz.
            warm_ps = warmpool.tile([1, 512], f32, name="warm_ps")

            def warm(n):
                for _ in range(n):
                    nc.tensor.matmul(warm_ps, warm16, k16[:, 0, 0:512],
                                     start=True, stop=True)

            ag_in = dram.tile([BLK, 9 * C], f32, name="ag_in")
            ag_out = dram.tile([BLK * NCORES, 9 * C], f32, name="ag_out")

            nc.sync.dma_start(out=g_sb, in_=g_dram[:, :])
            nc.sync.dma_start(out=h_sb, in_=h_dram[:, :])
            nc.sync.dma_start(out=ipp_sb, in_=ipp_dram[:, :])
            nc.sync.dma_start(out=icn_sb, in_=icn_dram[:, :])
            nc.sync.dma_start(out=boff_sb, in_=boff_dram[:, :])
            make_identity(nc, ident)
            nc.vector.memset(flat_pad, 0.0)

            # band offset registers (elements into flat_pad):
            #   off_l = (9r + PADBLK + BAND_LO)*C  -> left outer start (=45r)
            #   off_r = off_l + (CEN_LO + 9)*C     -> right outer start
            boff_regs = nc.alloc_registers("boff_regs",
                                           engines=(mybir.EngineType.DVE,))
            nc.regs_load(boff_regs, boff_sb[0:1, 0:1])
            off_l = nc.snap(boff_regs, donate=True, min_val=0,
                            max_val=(NCORES - 1) * 9 * C)
            boff2_regs = nc.alloc_registers("boff2_regs",
                                            engines=(mybir.EngineType.DVE,))
            nc.regs_load(boff2_regs, boff_sb[0:1, 1:2])
            off_r = nc.snap(boff2_regs, donate=True, min_val=(CEN_LO + 9) * C,
                            max_val=(NCORES - 1) * 9 * C + (CEN_LO + 9) * C)

            # ---- phase 1: build K band (all fp16) ----
            # Bilateral part on device (input-dependent); the gaussian part is
            # input-independent so the host ships it precomputed (kg16).
            with tc.tile_pool(name="kgstage", bufs=3) as kgpool, \
                 tc.tile_pool(name="bpsum", bufs=2, space="PSUM") as bppool:
                for i in range(NBLK):
                    kg = kgpool.tile([BLK, NLOC], f16, tag="kg")
                    nc.sync.dma_start(
                        out=kg, in_=kg_dram[:, i * NLOC:(i + 1) * NLOC])
                    pb = bppool.tile([BLK, NCH, 512], f32, tag="pb")
                    gt = g_sb[:, i * BLK:(i + 1) * BLK]
                    for nb in range(NCH):
                        hs = h_sb[:, nb * CH:(nb + 1) * CH]
                        nc.tensor.matmul(pb[:, nb, 0:CH], gt[0:9, :], hs[0:9, :],
                                         start=True, stop=True)
                    kdst = k16[:, i, :]
                    kv = kdst.rearrange("p (a c) -> p a c", c=CH)
                    nc.scalar.activation(out=kv, in_=pb[:, :, 0:CH], func=EXP,
                                         bias=ln3_sb)
                    nc.vector.tensor_add(kdst, kdst, kg)

            # ---- helpers ----
            def softmax_pp(pool, u_pp, mb, tag):
                """u_pp: [128, mb*C] logits, pixel-partition layout -> probs."""
                v = u_pp.rearrange("p (a c) -> p a c", c=C)
                mx = pool.tile([BLK, mb], f32, tag=f"{tag}_mx")
                nc.vector.tensor_reduce(out=mx, in_=v,
                                        axis=mybir.AxisListType.X,
                                        op=mybir.AluOpType.max)
                e = pool.tile([BLK, mb * C], f32, tag=f"{tag}_e")
                ev = e.rearrange("p (a c) -> p a c", c=C)
                nc.vector.tensor_sub(ev, v, bcast_inner(mx, C))
                nc.scalar.activation(out=e, in_=e, func=EXP)
                s = pool.tile([BLK, mb], f32, tag=f"{tag}_s")
                nc.vector.tensor_reduce(out=s, in_=ev,
                                        axis=mybir.AxisListType.X,
                                        op=mybir.AluOpType.add)
                nc.vector.reciprocal(out=s, in_=s)
                fl = pool.tile([BLK, mb * C], f32, tag=f"{tag}_fl")
                nc.vector.tensor_mul(fl.rearrange("p (a c) -> p a c", c=C), ev,
                                     bcast_inner(s, C))
                return fl

            # ---- phase 2: initial flat = softmax(input) ----
            with tc.tile_pool(name="init", bufs=1) as ipool:
                fl0 = softmax_pp(ipool, ipp_sb, GBLK, "sm0")
                nc.vector.tensor_copy(
                    out=flat_pad[:, PADBLK * C:(PADBLK + GBLK) * C], in_=fl0)

            # ---- phase 3: iterations ----
            with tc.tile_pool(name="iter", bufs=1) as wpool, \
                 tc.tile_pool(name="band", bufs=2) as bpool, \
                 tc.tile_pool(name="smx", bufs=2) as spool, \
                 tc.tile_pool(name="fl16p", bufs=2) as flpool, \
                 tc.tile_pool(name="ipsum", bufs=2, space="PSUM") as ippool:
                fl16 = None
                for it in range(ITERS):
                    # fp16 lhsT band. iter 0: whole band from local init
                    # softmax. later: central blocks direct from fl16 (local
                    # softmax), outer blocks from the AllGather.
                    band16 = bpool.tile([BLK, NBLK * C], f16, tag="band16")
                    if it == 0:
                        nc.vector.tensor_copy(
                            out=band16,
                            in_=flat_pad[:, bass.ds(off_l, NBLK * C)])
                    else:
                        nc.vector.tensor_copy(
                            out=band16[:, 0:CEN_LO * C],
                            in_=flat_pad[:, bass.ds(off_l, CEN_LO * C)])
                        nc.vector.tensor_copy(
                            out=band16[:, (CEN_LO + 9) * C:NBLK * C],
                            in_=flat_pad[:, bass.ds(off_r,
                                                    (NBLK - CEN_LO - 9) * C)])

                    # matvec: comb[c, n] = sum_m K[m, n] * flat[c, m]
                    # central (own 9 blocks) first: their lhsT (fl16) is ready
                    # before the AllGather lands, so the PE keeps working.
                    if it == 0:
                        order = list(range(NBLK))
                    else:
                        order = ([CEN_LO + j for j in range(9)]
                                 + [i for i in range(NBLK)
                                    if not CEN_LO <= i < CEN_LO + 9])
                    pv = ippool.tile([C, NCH, 512], f32, tag="pv", bufs=1)
                    for idx, i in enumerate(order):
                        if it > 0 and CEN_LO <= i < CEN_LO + 9:
                            lhs = fl16[:, (i - CEN_LO) * C:(i - CEN_LO + 1) * C]
                        else:
                            lhs = band16[:, i * C:(i + 1) * C]
                        for nb in range(NCH):
                            nc.tensor.matmul(
                                pv[:, nb, 0:CH], lhs,
                                k16[:, i, nb * CH:(nb + 1) * CH],
                                start=(idx == 0), stop=(idx == NBLK - 1))
                    comb = wpool.tile([C, NLOC], f32, tag="comb")
                    nc.scalar.activation(
                        out=comb.rearrange("p (a c) -> p a c", c=CH),
                        in_=pv[:, :, 0:CH], func=COPY)

                    # 3x3 box sum: x-pass into t1 (all 14 rows), edge-replicated
                    t1 = wpool.tile([C, NLOC], f32, tag="t1")
                    nc.vector.tensor_add(t1[:, 1:NLOC - 1], comb[:, 0:NLOC - 2],
                                         comb[:, 2:NLOC])
                    nc.vector.tensor_add(t1[:, 1:NLOC - 1], t1[:, 1:NLOC - 1],
                                         comb[:, 1:NLOC - 1])
                    t1r = t1.rearrange("p (row x) -> p row x", x=W)
                    cbr = comb.rearrange("p (row x) -> p row x", x=W)
                    # x = 0 column: 2*c[0] + c[1]
                    nc.vector.tensor_add(t1r[:, :, 0:1], cbr[:, :, 0:1],
                                         cbr[:, :, 1:2])
                    nc.vector.tensor_add(t1r[:, :, 0:1], t1r[:, :, 0:1],
                                         cbr[:, :, 0:1])
                    # x = W-1 column: c[W-2] + 2*c[W-1]
                    nc.vector.tensor_add(t1r[:, :, W - 1:W], cbr[:, :, W - 2:W - 1],
                                         cbr[:, :, W - 1:W])
                    nc.vector.tensor_add(t1r[:, :, W - 1:W], t1r[:, :, W - 1:W],
                                         cbr[:, :, W - 1:W])
                    # y-pass (middle 12 rows)
                    u = wpool.tile([C, NMID], f32, tag="u")
                    nc.vector.tensor_add(u, t1[:, 0:NMID], t1[:, 2 * W:NLOC])
                    nc.vector.tensor_add(u, u, t1[:, W:NMID + W])

                    # transpose U [5, 1152] -> pixel-partition [128, 9*5]:
                    # all 9 PE transposes into one PSUM tile, one DVE copy out.
                    pt = ippool.tile([BLK, 9 * C], f32, tag="pt")
                    for kb in range(9):
                        nc.tensor.transpose(pt[:, kb * C:(kb + 1) * C],
                                            u[:, kb * BLK:(kb + 1) * BLK],
                                            ident[0:C, 0:C])
                    u_pp = spool.tile([BLK, 9 * C], f32, tag="u_pp")
                    nc.vector.tensor_add(u_pp, pt, icn_sb)

                    flat_l = softmax_pp(spool, u_pp, 9, "smx")
                    fl16 = flpool.tile([BLK, 9 * C], f16, tag="fl16")
                    nc.vector.tensor_copy(out=fl16, in_=flat_l)

                    if it < ITERS - 1:
                        nc.sync.dma_start(out=ag_in, in_=flat_l)
                        nc.gpsimd.collective_compute(
                            "AllGather",
                            mybir.AluOpType.bypass,
                            replica_groups=[list(range(NCORES))],
                            ins=[ag_in.opt()],
                            outs=[ag_out.opt()],
                        )
                        warm(14)
                        nc.sync.dma_start(
                            out=flat_pad[:, PADBLK * C:(PADBLK + GBLK) * C]
                            .rearrange("p (r j) -> p r j", r=NCORES),
                            in_=ag_out.rearrange("(r p) j -> p r j", p=BLK))
                    else:
                        nc.sync.dma_start(out=out_dram[:, :], in_=flat_l)

    nc.compile()
    return nc


def _host_inputs(input_tensor, reference_tensor):
    logits = np.ascontiguousarray(
        np.asarray(input_tensor, dtype=np.float32)[0].reshape(C, N))
    ref = np.asarray(reference_tensor, dtype=np.float32)[0]  # [3, 96, 96]

    yy, xx = np.meshgrid(np.arange(H, dtype=np.float32),
                         np.arange(W, dtype=np.float32), indexing="ij")
    Y = (yy / 5.0).reshape(N)
    X = (xx / 5.0).reshape(N)
    RGB = (ref / 0.5).reshape(3, N)
    s2 = -0.5 * (Y * Y + X * X)
    c2 = -0.5 * (RGB * RGB).sum(axis=0)
    ones = np.ones(N, np.float32)

    # G (band / m side) and H (output / n side) augmented features
    G_all = np.stack([Y, X, s2, ones, RGB[0], RGB[1], RGB[2], c2, ones])
    H_all = np.stack([Y, X, ones, s2, RGB[0], RGB[1], RGB[2], ones, c2])

    # input in pixel-partition layout [128, 72*5]
    ipp = np.ascontiguousarray(
        logits.reshape(C, GBLK, BLK).transpose(2, 1, 0).reshape(BLK, GBLK * C))

    # gaussian kernel tables: 3*exp(-(dy^2+dx^2)/50), folded update factor 3
    dtab = np.exp(-(np.arange(-(H - 1), H) ** 2) / 50.0).astype(np.float64)
    gx3 = (3.0 * dtab).astype(np.float32)
    gy1 = dtab.astype(np.float32)
    yy_all = (np.arange(N) // W).astype(np.int64)
    xx_all = (np.arange(N) % W).astype(np.int64)

    def kg_for_core(r, yn, xn):
        """[128, NBLK*1344] fp16 gaussian kernel values for core r's band."""
        kg = np.zeros((NBLK, BLK, NLOC), np.float32)
        for i in range(NBLK):
            gb = 9 * r + BAND_LO + i
            if 0 <= gb < GBLK:
                pm = np.arange(gb * BLK, (gb + 1) * BLK)
                A = gy1[yy_all[pm][:, None] - yn[None, :] + H - 1]
                B = gx3[xx_all[pm][:, None] - xn[None, :] + H - 1]
                kg[i] = A * B
        return np.ascontiguousarray(
            kg.transpose(1, 0, 2).reshape(BLK, NBLK * NLOC)).astype(np.float16)

    in_maps = []
    kg_interior = None
    for r in range(NCORES):
        g = np.zeros((9, NBLK * BLK), np.float32)
        g[2, :] = NEG
        for i in range(NBLK):
            gb = 9 * r + BAND_LO + i
            if 0 <= gb < GBLK:
                g[:, i * BLK:(i + 1) * BLK] = G_all[:, gb * BLK:(gb + 1) * BLK]
        yext = np.clip(np.arange(RPC * r - 1, RPC * (r + 1) + 1), 0, H - 1)
        hpix = (yext[:, None] * W + np.arange(W)[None, :]).reshape(-1)
        h = np.ascontiguousarray(H_all[:, hpix])
        icn = logits.reshape(C, H, W)[:, RPC * r:RPC * (r + 1), :].reshape(C, NMID)
        icn_pp = np.ascontiguousarray(
            icn.reshape(C, 9, BLK).transpose(2, 1, 0).reshape(BLK, 9 * C))
        # gaussian part of K (interior cores share one array)
        if 2 <= r <= 5:
            if kg_interior is None:
                kg_interior = kg_for_core(r, yy_all[hpix], xx_all[hpix])
            kg = kg_interior
        else:
            kg = kg_for_core(r, yy_all[hpix], xx_all[hpix])
        in_maps.append({
            "g_feats": g,
            "h_feats": h,
            "inp_pp": ipp,
            "icn_pp": icn_pp,
            "band_off": np.array([[9 * C * r,
                                   9 * C * r + (CEN_LO + 9) * C]], np.uint32),
            "kg16": kg,
        })
    return in_maps


def _assemble(results):
    out = np.empty((C, N), np.float32)
    for r in range(NCORES):
        blk = results[r]["out_loc"].reshape(BLK, 9, C)
        out[:, NMID * r:NMID * (r + 1)] = (
            blk.transpose(2, 1, 0).reshape(C, NMID))
    return out.reshape(1, C, H, W)


def _get_nc():
    global _CACHED_NC
    if _CACHED_NC is None:
        _CACHED_NC = _build_module()
    return _CACHED_NC


def run(input_tensor, reference_tensor, trace=False):
    from concourse.bass_utils import run_bass_kernel_spmd
    nc = _get_nc()
    in_maps = _host_inputs(input_tensor, reference_tensor)
    res = run_bass_kernel_spmd(nc, in_maps, core_ids=list(range(NCORES)),
                               trace=trace)
    return _assemble(res.results), res


def kernel(input_tensor, reference_tensor):
    out, _ = run(input_tensor, reference_tensor, trace=False)
    return out


# revision 14
# speedup vs baseline: 1.9773x; 1.3468x over previous
"""Dense-CRF mean-field inference on 8 Trainium2 NeuronCores.

Math restructuring (validated numerically against the jax reference):
  - Kb + Kg share weight 1.0 -> single kernel matrix K = exp(-.5 d2_b) + exp(-.5 d2_g).
  - The Potts 3x3 conv update is  upd[c] = boxsum3(S) - boxsum3(comb[c]) with
    S = sum_c comb[c]; the S part is class-independent so softmax drops it:
        out = softmax(input + UPDATE_FACTOR * boxsum3(comb[c])).
    The UPDATE_FACTOR (3.0) is folded into K via exp(x + ln 3).
  - Spatial sigma 5 -> K decays fast with |dy|; each core keeps a 41-block
    (5248 px) band of K rows resident in SBUF, all fp16 (validated err 1.1e-4
    on the real inputs; fp32 matmuls run dual-pass LOW_HIGH at 2.3x the cost,
    so fp16 everywhere doubles TensorE throughput).
  - -0.5*||fi-fj||^2 is computed by ONE matmul per kernel via augmented
    features: G=[y,x,-.5|s|^2,1,r,g,b,-.5|c|^2,1], H=[y,x,1,-.5|s|^2,r,g,b,1,-.5|c|^2];
    the gaussian part is input-independent and host-shipped in fp16.
  - Each core computes comb for 14 image rows (its 12 + 1 halo row each side,
    edge rows duplicated via clamped features) so the 3x3 conv is local.
    One AllGather of the new per-core probabilities per iteration; the next
    iteration's matvec runs its 9 central (own-pixel) K blocks first straight
    off the local softmax output, hiding most of the AllGather latency.

Sharding: core r owns output image rows [12r, 12r+12); K band = global
128-px blocks [9r-16, 9r+25) (zero-K padding outside the image).
"""

import os
import sys

import numpy as np

for _p in ("/opt/trn_rl_repo",):
    if _p not in sys.path and os.path.isdir(_p):
        sys.path.insert(0, _p)

H = 96
W = 96
C = 5
N = H * W                      # 9216
NCORES = 8
RPC = H // NCORES              # 12 image rows per core
NLOC = (RPC + 2) * W           # 1344 extended-output pixels (14 rows)
NMID = RPC * W                 # 1152 owned pixels
BLK = 128
NBLK = 37                      # K band m-blocks per core
BAND_LO = -14                  # band start, in global blocks, relative to 9r
CEN_LO = -BAND_LO              # band-local index of first central (own) block
GBLK = N // BLK                # 72 global blocks
PADBLK = -BAND_LO              # padding blocks each side of flat_padded
FPW = (GBLK + 2 * PADBLK) * C  # flat_padded free width
CH = 448                       # matvec/exp n-chunk (fits one PSUM bank)
NCH = 3
ITERS = 5
LN3 = float(np.log(3.0))
NEG = -60000.0                 # kills exp() for padding blocks (fp16-safe)

_CACHED_NC = None


def _build_module():
    import concourse.bass as bass
    import concourse.bacc as bacc
    import concourse.tile as tile
    from concourse import mybir
    from concourse.masks import make_identity

    f32 = mybir.dt.float32
    f16 = mybir.dt.float16
    u32 = mybir.dt.uint32
    EXP = mybir.ActivationFunctionType.Exp
    COPY = mybir.ActivationFunctionType.Copy

    nc = bacc.Bacc("TRN2", target_bir_lowering=False, debug=False,
                   num_devices=NCORES)

    ghi_dram = nc.dram_tensor("g_hi", [9, NBLK * BLK], f16, kind="ExternalInput")
    glo_dram = nc.dram_tensor("g_lo", [9, NBLK * BLK], f16, kind="ExternalInput")
    hhi_dram = nc.dram_tensor("h_hi", [9, NLOC], f16, kind="ExternalInput")
    hlo_dram = nc.dram_tensor("h_lo", [9, NLOC], f16, kind="ExternalInput")
    ipp_dram = nc.dram_tensor("inp_pp", [BLK, GBLK * C], f32, kind="ExternalInput")
    icn_dram = nc.dram_tensor("icn_pp", [BLK, 9 * C], f32, kind="ExternalInput")
    boff_dram = nc.dram_tensor("band_off", [1, 2], u32, kind="ExternalInput")
    kg_dram = nc.dram_tensor("kg16", [BLK, NBLK * NLOC], f16, kind="ExternalInput")
    out_dram = nc.dram_tensor("out_loc", [BLK, 9 * C], f32, kind="ExternalOutput")

    def bcast_inner(ap, n):
        return bass.AP(tensor=ap.tensor, offset=ap.offset, ap=[*ap.ap, [0, n]])

    with tile.TileContext(nc) as tc:
        with tc.tile_pool(name="singles", bufs=1) as singles, \
             tc.tile_pool(name="warmps", bufs=1, space="PSUM") as warmpool, \
             tc.tile_pool(name="dram", bufs=1, space="DRAM") as dram:

            # ---- long-lived SBUF state ----
            k16 = singles.tile([BLK, NBLK, NCH * CH], f16, name="k16")
            flat_pad = singles.tile([BLK, FPW], f32, name="flat_pad")
            ghi_sb = singles.tile([9, NBLK * BLK], f16, name="ghi_sb")
            glo_sb = singles.tile([9, NBLK * BLK], f16, name="glo_sb")
            hhi_sb = singles.tile([9, NLOC], f16, name="hhi_sb")
            hlo_sb = singles.tile([9, NLOC], f16, name="hlo_sb")
            ipp_sb = singles.tile([BLK, GBLK * C], f32, name="ipp_sb")
            icn_sb = singles.tile([BLK, 9 * C], f32, name="icn_sb")
            ident = singles.tile([BLK, BLK], f32, name="ident")
            boff_sb = singles.tile([1, 2], u32, name="boff_sb")
            ln3_sb = singles.tile([BLK, 1], f32, name="ln3_sb")
            nc.vector.memset(ln3_sb, LN3)
            # HAM warm-keeper: dummy fp16 matmuls bridge PE-idle windows so the
            # activity monitor keeps the PE clock at 2.4 GHz. Reading lhs from
            # fl16 makes them schedule into the AllGather window.
            warm_ps = warmpool.tile([1, 512], f32, name="warm_ps")

            def warm(n, lhs):
                for _ in range(n):
                    nc.tensor.matmul(warm_ps, lhs, k16[:, 0, 0:512],
                                     start=True, stop=True)

            ag_in = dram.tile([BLK, 9 * C], f32, name="ag_in")
            ag_out = dram.tile([BLK * NCORES, 9 * C], f32, name="ag_out")

            nc.sync.dma_start(out=ghi_sb, in_=ghi_dram[:, :])
            nc.sync.dma_start(out=glo_sb, in_=glo_dram[:, :])
            nc.sync.dma_start(out=hhi_sb, in_=hhi_dram[:, :])
            nc.sync.dma_start(out=hlo_sb, in_=hlo_dram[:, :])
            nc.sync.dma_start(out=ipp_sb, in_=ipp_dram[:, :])
            nc.sync.dma_start(out=icn_sb, in_=icn_dram[:, :])
            nc.sync.dma_start(out=boff_sb, in_=boff_dram[:, :])
            make_identity(nc, ident)
            nc.vector.memset(flat_pad, 0.0)

            # band offset registers (elements into flat_pad):
            #   off_l = (9r + PADBLK + BAND_LO)*C  -> left outer start (=45r)
            #   off_r = off_l + (CEN_LO + 9)*C     -> right outer start
            boff_regs = nc.alloc_registers("boff_regs",
                                           engines=(mybir.EngineType.DVE,))
            nc.regs_load(boff_regs, boff_sb[0:1, 0:1])
            off_l = nc.snap(boff_regs, donate=True, min_val=0,
                            max_val=(NCORES - 1) * 9 * C)
            boff2_regs = nc.alloc_registers("boff2_regs",
                                            engines=(mybir.EngineType.DVE,))
            nc.regs_load(boff2_regs, boff_sb[0:1, 1:2])
            off_r = nc.snap(boff2_regs, donate=True, min_val=(CEN_LO + 9) * C,
                            max_val=(NCORES - 1) * 9 * C + (CEN_LO + 9) * C)

            # ---- phase 1: build K band (all fp16) ----
            # Bilateral part on device (input-dependent); the gaussian part is
            # input-independent so the host ships it precomputed (kg16).
            with tc.tile_pool(name="kgstage", bufs=3) as kgpool, \
                 tc.tile_pool(name="bpsum", bufs=2, space="PSUM") as bppool:
                for i in range(NBLK):
                    kg = kgpool.tile([BLK, NLOC], f16, tag="kg")
                    nc.sync.dma_start(
                        out=kg, in_=kg_dram[:, i * NLOC:(i + 1) * NLOC])
                    pb = bppool.tile([BLK, NCH, 512], f32, tag="pb")
                    ghi = ghi_sb[0:9, i * BLK:(i + 1) * BLK]
                    glo = glo_sb[0:9, i * BLK:(i + 1) * BLK]
                    # fp16 hi/lo emulation of the fp32 feature product:
                    # pre = ghi.hhi + ghi.hlo + glo.hhi (lo.lo term ~1e-7)
                    for nb in range(NCH):
                        sl = slice(nb * CH, (nb + 1) * CH)
                        nc.tensor.matmul(pb[:, nb, 0:CH], ghi, hhi_sb[0:9, sl],
                                         start=True, stop=False)
                        nc.tensor.matmul(pb[:, nb, 0:CH], ghi, hlo_sb[0:9, sl],
                                         start=False, stop=False)
                        nc.tensor.matmul(pb[:, nb, 0:CH], glo, hhi_sb[0:9, sl],
                                         start=False, stop=True)
                    kdst = k16[:, i, :]
                    kv = kdst.rearrange("p (a c) -> p a c", c=CH)
                    nc.scalar.activation(out=kv, in_=pb[:, :, 0:CH], func=EXP,
                                         bias=ln3_sb)
                    nc.vector.tensor_add(kdst, kdst, kg)

            # ---- helpers ----
            def softmax_pp(pool, u_pp, mb, tag):
                """u_pp: [128, mb*C] logits, pixel-partition layout -> probs."""
                v = u_pp.rearrange("p (a c) -> p a c", c=C)
                mx = pool.tile([BLK, mb], f32, tag=f"{tag}_mx")
                nc.vector.tensor_reduce(out=mx, in_=v,
                                        axis=mybir.AxisListType.X,
                                        op=mybir.AluOpType.max)
                e = pool.tile([BLK, mb * C], f32, tag=f"{tag}_e")
                ev = e.rearrange("p (a c) -> p a c", c=C)
                nc.vector.tensor_sub(ev, v, bcast_inner(mx, C))
                nc.scalar.activation(out=e, in_=e, func=EXP)
                s = pool.tile([BLK, mb], f32, tag=f"{tag}_s")
                nc.vector.tensor_reduce(out=s, in_=ev,
                                        axis=mybir.AxisListType.X,
                                        op=mybir.AluOpType.add)
                nc.vector.reciprocal(out=s, in_=s)
                fl = pool.tile([BLK, mb * C], f32, tag=f"{tag}_fl")
                nc.vector.tensor_mul(fl.rearrange("p (a c) -> p a c", c=C), ev,
                                     bcast_inner(s, C))
                return fl

            # ---- phase 2: initial flat = softmax(input) ----
            with tc.tile_pool(name="init", bufs=1) as ipool:
                fl0 = softmax_pp(ipool, ipp_sb, GBLK, "sm0")
                nc.vector.tensor_copy(
                    out=flat_pad[:, PADBLK * C:(PADBLK + GBLK) * C], in_=fl0)

            # ---- phase 3: iterations ----
            with tc.tile_pool(name="iter", bufs=1) as wpool, \
                 tc.tile_pool(name="band", bufs=2) as bpool, \
                 tc.tile_pool(name="smx", bufs=2) as spool, \
                 tc.tile_pool(name="fl16p", bufs=2) as flpool, \
                 tc.tile_pool(name="ipsum", bufs=2, space="PSUM") as ippool:
                fl16 = None
                for it in range(ITERS):
                    # fp16 lhsT band. iter 0: whole band from local init
                    # softmax. later: central blocks direct from fl16 (local
                    # softmax), outer blocks from the AllGather.
                    band16 = bpool.tile([BLK, NBLK * C], f16, tag="band16")
                    if it == 0:
                        nc.vector.tensor_copy(
                            out=band16,
                            in_=flat_pad[:, bass.ds(off_l, NBLK * C)])
                    else:
                        nc.vector.tensor_copy(
                            out=band16[:, 0:CEN_LO * C],
                            in_=flat_pad[:, bass.ds(off_l, CEN_LO * C)])
                        nc.vector.tensor_copy(
                            out=band16[:, (CEN_LO + 9) * C:NBLK * C],
                            in_=flat_pad[:, bass.ds(off_r,
                                                    (NBLK - CEN_LO - 9) * C)])

                    # matvec: comb[c, n] = sum_m K[m, n] * flat[c, m]
                    # central (own 9 blocks) first: their lhsT (fl16) is ready
                    # before the AllGather lands, so the PE keeps working.
                    if it == 0:
                        order = list(range(NBLK))
                    else:
                        order = ([CEN_LO + j for j in range(9)]
                                 + [i for i in range(NBLK)
                                    if not CEN_LO <= i < CEN_LO + 9])
                    pv = ippool.tile([C, NCH, 512], f32, tag="pv", bufs=1)
                    for idx, i in enumerate(order):
                        if it > 0 and CEN_LO <= i < CEN_LO + 9:
                            lhs = fl16[:, (i - CEN_LO) * C:(i - CEN_LO + 1) * C]
                        else:
                            lhs = band16[:, i * C:(i + 1) * C]
                        for nb in range(NCH):
                            nc.tensor.matmul(
                                pv[:, nb, 0:CH], lhs,
                                k16[:, i, nb * CH:(nb + 1) * CH],
                                start=(idx == 0), stop=(idx == NBLK - 1))
                    comb = wpool.tile([C, NLOC], f32, tag="comb")
                    nc.scalar.activation(
                        out=comb.rearrange("p (a c) -> p a c", c=CH),
                        in_=pv[:, :, 0:CH], func=COPY)

                    # 3x3 box sum. y-pass first (14 rows -> 12, narrower x-pass
                    # after), each big add split across DVE and GpSimd.
                    XS = 768
                    ty = wpool.tile([C, NMID], f32, tag="ty")
                    nc.vector.tensor_add(ty[:, 0:XS], comb[:, 0:XS],
                                         comb[:, 2 * W:2 * W + XS])
                    nc.gpsimd.tensor_add(ty[:, XS:NMID], comb[:, XS:NMID],
                                         comb[:, 2 * W + XS:NLOC])
                    nc.vector.tensor_add(ty[:, 0:XS], ty[:, 0:XS],
                                         comb[:, W:W + XS])
                    nc.gpsimd.tensor_add(ty[:, XS:NMID], ty[:, XS:NMID],
                                         comb[:, W + XS:W + NMID])
                    # x-pass: contiguous shifted adds, then fix edge columns
                    u = wpool.tile([C, NMID], f32, tag="u")
                    nc.vector.tensor_add(u[:, 1:XS], ty[:, 0:XS - 1],
                                         ty[:, 2:XS + 1])
                    nc.gpsimd.tensor_add(u[:, XS:NMID - 1], ty[:, XS - 1:NMID - 2],
                                         ty[:, XS + 1:NMID])
                    nc.vector.tensor_add(u[:, 1:XS], u[:, 1:XS], ty[:, 1:XS])
                    nc.gpsimd.tensor_add(u[:, XS:NMID - 1], u[:, XS:NMID - 1],
                                         ty[:, XS:NMID - 1])
                    ur = u.rearrange("p (row x) -> p row x", x=W)
                    tyr = ty.rearrange("p (row x) -> p row x", x=W)
                    # x = 0 column: 2*t[0] + t[1]
                    nc.vector.tensor_add(ur[:, :, 0:1], tyr[:, :, 0:1],
                                         tyr[:, :, 1:2])
                    nc.vector.tensor_add(ur[:, :, 0:1], ur[:, :, 0:1],
                                         tyr[:, :, 0:1])
                    # x = W-1 column: t[W-2] + 2*t[W-1]
                    nc.vector.tensor_add(ur[:, :, W - 1:W], tyr[:, :, W - 2:W - 1],
                                         tyr[:, :, W - 1:W])
                    nc.vector.tensor_add(ur[:, :, W - 1:W], ur[:, :, W - 1:W],
                                         tyr[:, :, W - 1:W])

                    # transpose U [5, 1152] -> pixel-partition [128, 9*5]:
                    # all 9 PE transposes into one PSUM tile, one DVE copy out.
                    pt = ippool.tile([BLK, 9 * C], f32, tag="pt")
                    for kb in range(9):
                        nc.tensor.transpose(pt[:, kb * C:(kb + 1) * C],
                                            u[:, kb * BLK:(kb + 1) * BLK],
                                            ident[0:C, 0:C])
                    u_pp = spool.tile([BLK, 9 * C], f32, tag="u_pp")
                    nc.vector.tensor_add(u_pp, pt, icn_sb)

                    flat_l = softmax_pp(spool, u_pp, 9, "smx")
                    fl16 = flpool.tile([BLK, 9 * C], f16, tag="fl16")
                    nc.vector.tensor_copy(out=fl16, in_=flat_l)

                    if it < ITERS - 1:
                        nc.sync.dma_start(out=ag_in, in_=flat_l)
                        nc.gpsimd.collective_compute(
                            "AllGather",
                            mybir.AluOpType.bypass,
                            replica_groups=[list(range(NCORES))],
                            ins=[ag_in.opt()],
                            outs=[ag_out.opt()],
                        )
                        warm(24, fl16[:, 0:1])
                        nc.sync.dma_start(
                            out=flat_pad[:, PADBLK * C:(PADBLK + GBLK) * C]
                            .rearrange("p (r j) -> p r j", r=NCORES),
                            in_=ag_out.rearrange("(r p) j -> p r j", p=BLK))
                    else:
                        nc.sync.dma_start(out=out_dram[:, :], in_=flat_l)

    nc.compile()
    return nc


def _host_inputs(input_tensor, reference_tensor):
    logits = np.ascontiguousarray(
        np.asarray(input_tensor, dtype=np.float32)[0].reshape(C, N))
    ref = np.asarray(reference_tensor, dtype=np.float32)[0]  # [3, 96, 96]

    yy, xx = np.meshgrid(np.arange(H, dtype=np.float32),
                         np.arange(W, dtype=np.float32), indexing="ij")
    Y = (yy / 5.0).reshape(N)
    X = (xx / 5.0).reshape(N)
    RGB = (ref / 0.5).reshape(3, N)
    s2 = -0.5 * (Y * Y + X * X)
    c2 = -0.5 * (RGB * RGB).sum(axis=0)
    ones = np.ones(N, np.float32)

    # G (band / m side) and H (output / n side) augmented features
    G_all = np.stack([Y, X, s2, ones, RGB[0], RGB[1], RGB[2], c2, ones])
    H_all = np.stack([Y, X, ones, s2, RGB[0], RGB[1], RGB[2], ones, c2])

    # input in pixel-partition layout [128, 72*5]
    ipp = np.ascontiguousarray(
        logits.reshape(C, GBLK, BLK).transpose(2, 1, 0).reshape(BLK, GBLK * C))

    # gaussian kernel tables: 3*exp(-(dy^2+dx^2)/50), folded update factor 3
    dtab = np.exp(-(np.arange(-(H - 1), H) ** 2) / 50.0).astype(np.float64)
    gx3 = (3.0 * dtab).astype(np.float32)
    gy1 = dtab.astype(np.float32)
    yy_all = (np.arange(N) // W).astype(np.int64)
    xx_all = (np.arange(N) % W).astype(np.int64)

    def kg_for_core(r, yn, xn):
        """[128, NBLK*1344] fp16 gaussian kernel values for core r's band."""
        kg = np.zeros((NBLK, BLK, NLOC), np.float32)
        for i in range(NBLK):
            gb = 9 * r + BAND_LO + i
            if 0 <= gb < GBLK:
                pm = np.arange(gb * BLK, (gb + 1) * BLK)
                A = gy1[yy_all[pm][:, None] - yn[None, :] + H - 1]
                B = gx3[xx_all[pm][:, None] - xn[None, :] + H - 1]
                kg[i] = A * B
        return np.ascontiguousarray(
            kg.transpose(1, 0, 2).reshape(BLK, NBLK * NLOC)).astype(np.float16)

    def hilo(a):
        hi = a.astype(np.float16)
        lo = (a - hi.astype(np.float32)).astype(np.float16)
        return np.ascontiguousarray(hi), np.ascontiguousarray(lo)

    in_maps = []
    kg_interior = None
    for r in range(NCORES):
        g = np.zeros((9, NBLK * BLK), np.float32)
        g[2, :] = NEG
        for i in range(NBLK):
            gb = 9 * r + BAND_LO + i
            if 0 <= gb < GBLK:
                g[:, i * BLK:(i + 1) * BLK] = G_all[:, gb * BLK:(gb + 1) * BLK]
        g_hi, g_lo = hilo(g)
        yext = np.clip(np.arange(RPC * r - 1, RPC * (r + 1) + 1), 0, H - 1)
        hpix = (yext[:, None] * W + np.arange(W)[None, :]).reshape(-1)
        h = np.ascontiguousarray(H_all[:, hpix])
        h_hi, h_lo = hilo(h)
        icn = logits.reshape(C, H, W)[:, RPC * r:RPC * (r + 1), :].reshape(C, NMID)
        icn_pp = np.ascontiguousarray(
            icn.reshape(C, 9, BLK).transpose(2, 1, 0).reshape(BLK, 9 * C))
        # gaussian part of K (interior cores share one array)
        if 2 <= r <= 5:
            if kg_interior is None:
                kg_interior = kg_for_core(r, yy_all[hpix], xx_all[hpix])
            kg = kg_interior
        else:
            kg = kg_for_core(r, yy_all[hpix], xx_all[hpix])
        in_maps.append({
            "g_hi": g_hi,
            "g_lo": g_lo,
            "h_hi": h_hi,
            "h_lo": h_lo,
            "inp_pp": ipp,
            "icn_pp": icn_pp,
            "band_off": np.array([[9 * C * r,
                                   9 * C * r + (CEN_LO + 9) * C]], np.uint32),
            "kg16": kg,
        })
    return in_maps


def _assemble(results):
    out = np.empty((C, N), np.float32)
    for r in range(NCORES):
        blk = results[r]["out_loc"].reshape(BLK, 9, C)
        out[:, NMID * r:NMID * (r + 1)] = (
            blk.transpose(2, 1, 0).reshape(C, NMID))
    return out.reshape(1, C, H, W)


def _get_nc():
    global _CACHED_NC
    if _CACHED_NC is None:
        _CACHED_NC = _build_module()
    return _CACHED_NC


def run(input_tensor, reference_tensor, trace=False):
    from concourse.bass_utils import run_bass_kernel_spmd
    nc = _get_nc()
    in_maps = _host_inputs(input_tensor, reference_tensor)
    res = run_bass_kernel_spmd(nc, in_maps, core_ids=list(range(NCORES)),
                               trace=trace)
    return _assemble(res.results), res


def kernel(input_tensor, reference_tensor):
    out, _ = run(input_tensor, reference_tensor, trace=False)
    return out
